# revision 5
# baseline (speedup 1.0000x reference)
"""Cross-attention kernel for Trainium2 (8 NeuronCores, data-parallel over batch).

Problem (hardcoded): B=8, Sq=4096, Sk=77, E=1024, C=768, H=16 heads, D=64.

    q = x @ wq + bq; k = y @ wk + bk; v = y @ wv + bv
    out = softmax(q k^T / sqrt(D)) v @ wo + bo

Sharding: batch element b -> core b. No collectives.

Per-core device pipeline (all matmuls contract over the SBUF partition dim):
  - Activations are kept feature-major ("transposed"): xT[E, Sq] is prepared
    host-side, so the wq-lhsT matmul chain produces qT[E, Sq] directly and
    per-head slices qT[h*64:(h+1)*64, :] feed scores without any on-chip
    transpose.
  - kT[E, Sk] via lhsT=wk tiles; V[Sk, E] via lhsT=yT tiles, then re-packed
    on-chip into the zero-padded pair layout v2[Sk, et, {[v_even|0],
    [0|v_odd]}] (zeros DMA'd from a host constant, values written by strided
    DVE evictions).
  - scores^T[Sk, q] = matmul(lhsT=kT head slice [64, 77], rhs=qT head slice).
    The 1/sqrt(D) scale is folded into wq/bq host-side.
  - softmax without max-subtraction (scores are O(5), exp is safe in fp32):
    exp on ScalarE.
  - Attention output for a head PAIR accumulates into one [128, 512] PSUM
    tile: matmul(lhsT=[v_even|0], rhs=exp_even) start, then
    matmul(lhsT=[0|v_odd], rhs=exp_odd) stop - rows 0:64 get the even head,
    64:128 the odd head, exactly the oT layout the final projection wants.
    Pair denominators accumulate the same way from ones-block lhsT operands
    into a second [128, 512] tile, arriving pre-broadcast across each
    64-partition half; the DVE fast-approx reciprocal evicts them to SBUF
    and one in-place multiply per pair (interleaved into the next chunk's
    QT phase) normalizes oT.
  - out[q, E] row-major = matmul(lhsT=oT tiles [128, 128], rhs=wo tiles),
    bias bo added during eviction from a partition-broadcast bias tile.

All matmul operands are typed float32r (fp32 with 11 mantissa bits): 1
cycle/row on the PE at N=512. Operands coming from DRAM are pre-rounded
host-side; on-chip producers round by writing float32r-typed outputs.
"""

import os
from contextlib import ExitStack

import numpy as np

import concourse.bass as bass
import concourse.tile as tile
from concourse import bacc, mybir
from concourse.bass_utils import run_bass_kernel_spmd

N_CORES = 8
SQ = 4096
SK = 77
SKP = 80  # SK padded: fp32r matmul dst free-size must be even
E = 1024
C = 768
H = 16
D = 64
CHUNK = 512
NCHUNK = SQ // CHUNK  # 8
ET = E // 128  # 8 e-tiles
CT = C // 128  # 6 c-tiles
F32 = mybir.dt.float32
F32R = mybir.dt.float32r

_PROGRAM = None


def _round_f32r(a: np.ndarray) -> np.ndarray:
    """Round fp32 to the fp32r format (11 mantissa bits, RNE)."""
    u = np.ascontiguousarray(a, dtype=np.float32).view(np.uint32).copy()
    u += np.uint32(0x7FF) + ((u >> np.uint32(12)) & np.uint32(1))
    u &= np.uint32(0xFFFFF000)
    return u.view(np.float32)


def _build_program():
    nc = bacc.Bacc(
        "TRN2", target_bir_lowering=False, debug=False, num_devices=N_CORES
    )
    # xT pre-tiled host-side: [chunk, partition, e-tile, col] so each chunk's
    # SBUF tile is one contiguous 2MB DRAM read (16KB per partition row).
    xT_d = nc.dram_tensor(
        "xT", [NCHUNK, 128, ET, CHUNK], F32R, kind="ExternalInput"
    ).ap()
    yT_d = nc.dram_tensor("yT", [C, SKP], F32R, kind="ExternalInput").ap()
    wq_d = nc.dram_tensor("wq", [E, E], F32R, kind="ExternalInput").ap()
    bq_d = nc.dram_tensor("bq", [E], F32, kind="ExternalInput").ap()
    wk_d = nc.dram_tensor("wk", [C, E], F32R, kind="ExternalInput").ap()
    bk_d = nc.dram_tensor("bk", [E], F32, kind="ExternalInput").ap()
    wv_d = nc.dram_tensor("wv", [C, H * 64], F32R, kind="ExternalInput").ap()
    bv_d = nc.dram_tensor("bv", [H * 64], F32, kind="ExternalInput").ap()
    wo_d = nc.dram_tensor("wo", [E, E], F32R, kind="ExternalInput").ap()
    bo_d = nc.dram_tensor("bo", [E], F32, kind="ExternalInput").ap()
    # ones blocks for pair denominators: [:, 0, 0:64] = 1 and [:, 1, 64:128] = 1
    ones_d = nc.dram_tensor("ones", [SK, 2 * 128], F32R, kind="ExternalInput").ap()
    z2_d = nc.dram_tensor("z2", [SK, ET * 2 * 128], F32R, kind="ExternalInput").ap()
    out_d = nc.dram_tensor("out", [SQ, E], F32, kind="ExternalOutput").ap()

    with tile.TileContext(nc) as tc, ExitStack() as ctx:
        consts = ctx.enter_context(tc.tile_pool(name="consts", bufs=1))
        wq_sb = consts.tile([128, ET, E], F32R)
        wo_sb = consts.tile([128, ET, E], F32R)
        kT_sb = consts.tile([128, ET, SKP], F32R)
        v2_sb = consts.tile([SK, ET, 2, 128], F32R)
        ones_sb = consts.tile([SK, 2, 128], F32R)
        bq_sb = consts.tile([128, ET], F32)
        bk_sb = consts.tile([128, ET], F32)
        bv_sb = consts.tile([SK, H * 64], F32)
        bo_sb = consts.tile([128, E], F32)

        # q-path loads first so QT(0) starts as early as possible.
        wq_r = wq_d.rearrange("(t p) n -> p t n", p=128)
        for lo, hi in ((0, 4), (4, 8)):
            nc.sync.dma_start(wq_sb[:, lo:hi, :], wq_r[:, lo:hi, :])
        nc.sync.dma_start(bq_sb[:], bq_d.rearrange("(t p) -> p t", p=128))

        xT_pool = ctx.enter_context(tc.tile_pool(name="xT", bufs=2))
        qT_pool = ctx.enter_context(tc.tile_pool(name="qT", bufs=1))
        ps_q = ctx.enter_context(tc.tile_pool(name="ps_q", bufs=2, space="PSUM"))

        def load_xT(c):
            xT_sb = xT_pool.tile([128, ET, CHUNK], F32R, tag="xT")
            nc.sync.dma_start(xT_sb[:], xT_d[c])
            return xT_sb

        xT_cur = load_xT(0)

        # Remaining const loads (needed by phase 0 / attention / finals).
        nc.sync.dma_start(ones_sb[:], ones_d.rearrange("p (s n) -> p s n", s=2))
        nc.sync.dma_start(
            v2_sb[:], z2_d.rearrange("p (t s n) -> p t s n", t=ET, s=2)
        )
        nc.sync.dma_start(bk_sb[:], bk_d.rearrange("(t p) -> p t", p=128))
        nc.sync.dma_start(bv_sb[:], bv_d.partition_broadcast(SK))
        nc.sync.dma_start(bo_sb[:], bo_d.partition_broadcast(128))

        def emit_qt(xT_sb, interleave=None):
            qT_sb = qT_pool.tile([128, ET, CHUNK], F32R, tag="qT", name="qT_sb")
            for et in range(ET):
                ps = ps_q.tile([128, CHUNK], F32, tag="psq")
                for t in range(ET):
                    nc.tensor.matmul(
                        ps[:],
                        wq_sb[:, t, et * 128 : (et + 1) * 128],
                        xT_sb[:, t, :],
                        start=(t == 0),
                        stop=(t == ET - 1),
                    )
                nc.scalar.activation(
                    qT_sb[:, et, :],
                    ps[:],
                    mybir.ActivationFunctionType.Identity,
                    bias=bq_sb[:, et : et + 1],
                )
                if interleave is not None:
                    interleave(et)
            return qT_sb

        # QT(0): emitted before phase 0 so the PE ramps on it while the
        # k/v-side weights stream in.
        qT_sb0 = emit_qt(xT_cur)

        # Phase 0: kT[E, Sk], V[Sk, E], and the padded pair layout v2.
        with tc.tile_pool(name="ph0", bufs=1) as ph0, tc.tile_pool(
            name="ph0ps", bufs=4, space="PSUM"
        ) as ph0ps:
            yT_sb = ph0.tile([128, CT, SKP], F32R)
            wk_sb = ph0.tile([128, CT, E], F32R)
            wv_sb = ph0.tile([128, CT, H * 64], F32R)
            v_sb = ph0.tile([SK, H * 64], F32R)
            yT_r = yT_d.rearrange("(t p) n -> p t n", p=128)
            wk_r = wk_d.rearrange("(t p) n -> p t n", p=128)
            wv_r = wv_d.rearrange("(t p) n -> p t n", p=128)
            nc.sync.dma_start(yT_sb[:], yT_r)
            for t in range(CT):
                nc.sync.dma_start(wk_sb[:, t, :], wk_r[:, t, :])
            for t in range(CT):
                nc.sync.dma_start(wv_sb[:, t, :], wv_r[:, t, :])
            for et in range(ET):
                psk = ph0ps.tile([128, SKP], F32, tag="ph0", name="psk")
                for t in range(CT):
                    nc.tensor.matmul(
                        psk[:],
                        wk_sb[:, t, et * 128 : (et + 1) * 128],
                        yT_sb[:, t, :],
                        start=(t == 0),
                        stop=(t == CT - 1),
                    )
                nc.scalar.activation(
                    kT_sb[:, et, :],
                    psk[:],
                    mybir.ActivationFunctionType.Identity,
                    bias=bk_sb[:, et : et + 1],
                )
            for g in range(2):
                psv = ph0ps.tile([SK, CHUNK], F32, tag="ph0", name="psv")
                for t in range(CT):
                    nc.tensor.matmul(
                        psv[:],
                        yT_sb[:, t, 0:SK],
                        wv_sb[:, t, g * CHUNK : (g + 1) * CHUNK],
                        start=(t == 0),
                        stop=(t == CT - 1),
                    )
                nc.vector.tensor_tensor(
                    v_sb[:, g * CHUNK : (g + 1) * CHUNK],
                    psv[:],
                    bv_sb[:, g * CHUNK : (g + 1) * CHUNK],
                    mybir.AluOpType.add,
                )
                # Re-pack group g (heads 8g..8g+7, ets 4g..4g+3) into the
                # padded pair layout with two strided copies: even heads to
                # slot 0 cols 0:64, odd heads to slot 1 cols 64:128.
                e0 = 4 * g
                v_g = v_sb[:, g * CHUNK : (g + 1) * CHUNK].rearrange(
                    "p (e s d) -> p e s d", e=4, s=2
                )
                with nc.allow_low_precision(reason="fp32r pass-through copy"):
                    nc.vector.tensor_copy(
                        v2_sb[:, e0 : e0 + 4, 0, 0:64], v_g[:, :, 0, :]
                    )
                    nc.vector.tensor_copy(
                        v2_sb[:, e0 : e0 + 4, 1, 64:128], v_g[:, :, 1, :]
                    )

        # Main loop pools.
        oT_pool = ctx.enter_context(tc.tile_pool(name="oT", bufs=2))
        rb_pool = ctx.enter_context(tc.tile_pool(name="rb", bufs=1))
        exps_pool = ctx.enter_context(tc.tile_pool(name="exps", bufs=4))
        outs_pool = ctx.enter_context(tc.tile_pool(name="outs", bufs=2))
        ps_s = ctx.enter_context(tc.tile_pool(name="ps_s", bufs=2, space="PSUM"))
        ps_av = ctx.enter_context(tc.tile_pool(name="ps_av", bufs=2, space="PSUM"))
        ps_den = ctx.enter_context(tc.tile_pool(name="ps_den", bufs=1, space="PSUM"))
        ps_f = ctx.enter_context(tc.tile_pool(name="ps_f", bufs=1, space="PSUM"))

        def emit_final_group(c, oT_sb, i):
            qt, n0 = i // 2, (i % 2) * CHUNK
            # Alternate between ps_f and the (idle during attention) ps_q
            # slots: double-buffers the final groups at zero PSUM-bank cost.
            if i % 2 == 0:
                ps = ps_f.tile([128, CHUNK], F32, tag="psf")
            else:
                ps = ps_q.tile([128, CHUNK], F32, tag="psq", name="psfq")
            for t in range(ET):
                nc.tensor.matmul(
                    ps[:],
                    oT_sb[:, t, qt * 128 : (qt + 1) * 128],
                    wo_sb[:, t, n0 : n0 + CHUNK],
                    start=(t == 0),
                    stop=(t == ET - 1),
                )
            o_sb = outs_pool.tile([128, CHUNK], F32, tag="osb")
            nc.vector.tensor_tensor(
                o_sb[:], ps[:], bo_sb[:, n0 : n0 + CHUNK], mybir.AluOpType.add
            )
            r0 = c * CHUNK + qt * 128
            nc.sync.dma_start(out_d[r0 : r0 + 128, n0 : n0 + CHUNK], o_sb[:])

        # wo is first needed by final(0) during attention(1); issuing its
        # load here keeps it off the QT(0)/phase-0 critical DMA path.
        wo_r = wo_d.rearrange("(t p) n -> p t n", p=128)
        for lo, hi in ((0, 4), (4, 8)):
            nc.sync.dma_start(wo_sb[:, lo:hi, :], wo_r[:, lo:hi, :])

        prev = None  # (c, oT_sb): chunk awaiting its final projection
        norm = None  # (oT_sb, rb_sb): chunk awaiting softmax normalization
        for c in range(NCHUNK):
            xT_sb = xT_cur
            if c + 1 < NCHUNK:
                xT_cur = load_xT(c + 1)
            # QT phase (skipped for c=0: already emitted). Chunk c-1's
            # normalization multiplies are interleaved between the QT groups
            # so the DVE work overlaps queued PE work.
            if c == 0:
                qT_sb = qT_sb0
            else:
                pnorm = norm

                def normalize(et, pnorm=pnorm):
                    if pnorm is not None:
                        nc.vector.tensor_tensor(
                            pnorm[0][:, et, :],
                            pnorm[0][:, et, :],
                            pnorm[1][:, et, :],
                            mybir.AluOpType.mult,
                        )

                qT_sb = emit_qt(xT_sb, interleave=normalize)
                norm = None

            # Attention for chunk c, interleaved with chunk c-1's output
            # projection. Per pair et: scores -> exp -> (finals cover the
            # exp latency) -> pair-packed attnV + pair-broadcast den ->
            # evict oT (ScalarE) + reciprocal to rb (DVE).
            oT_sb = oT_pool.tile([128, ET, CHUNK], F32R, tag="oT")
            rb_sb = rb_pool.tile([128, ET, CHUNK], F32, tag="rb")
            for et in range(ET):
                psa = ps_s.tile([SK, CHUNK], F32, tag="pss")
                psb = ps_s.tile([SK, CHUNK], F32, tag="pss")
                nc.tensor.matmul(
                    psa[:], kT_sb[0:64, et, 0:SK], qT_sb[0:64, et, :],
                    start=True, stop=True,
                )
                nc.tensor.matmul(
                    psb[:], kT_sb[64:128, et, 0:SK], qT_sb[64:128, et, :],
                    start=True, stop=True,
                )
                exa = exps_pool.tile([SK, CHUNK], F32R, tag="exps")
                exb = exps_pool.tile([SK, CHUNK], F32R, tag="exps")
                nc.scalar.activation(exa[:], psa[:], mybir.ActivationFunctionType.Exp)
                nc.scalar.activation(exb[:], psb[:], mybir.ActivationFunctionType.Exp)
                if prev is not None:
                    emit_final_group(prev[0], prev[1], et)
                pav = ps_av.tile([128, CHUNK], F32, tag="psav")
                den = ps_den.tile([128, CHUNK], F32, tag="psden")
                nc.tensor.matmul(
                    pav[:], v2_sb[:, et, 0, :], exa[:], start=True, stop=False
                )
                nc.tensor.matmul(
                    den[:], ones_sb[:, 0, :], exa[:], start=True, stop=False
                )
                nc.tensor.matmul(
                    pav[:], v2_sb[:, et, 1, :], exb[:], start=False, stop=True
                )
                nc.tensor.matmul(
                    den[:], ones_sb[:, 1, :], exb[:], start=False, stop=True
                )
                nc.scalar.activation(
                    oT_sb[:, et, :], pav[:], mybir.ActivationFunctionType.Identity
                )
                nc.vector.reciprocal_approx_fast(rb_sb[:, et, :], den[:])
            norm = (oT_sb, rb_sb)
            prev = (c, oT_sb)
        # Tail: normalize and project the last chunk.
        for et in range(ET):
            nc.vector.tensor_tensor(
                norm[0][:, et, :],
                norm[0][:, et, :],
                norm[1][:, et, :],
                mybir.AluOpType.mult,
            )
        for i in range(8):
            emit_final_group(prev[0], prev[1], i)

    nc.compile()
    return nc


def _get_program():
    global _PROGRAM
    if _PROGRAM is None:
        _PROGRAM = _build_program()
    return _PROGRAM


def kernel(x, y, wq, bq, wk, bk, wv, bv, wo, bo):
    x = np.asarray(x, dtype=np.float32)
    y = np.asarray(y, dtype=np.float32)
    wq = np.asarray(wq, dtype=np.float32)
    bq = np.asarray(bq, dtype=np.float32)
    wk = np.asarray(wk, dtype=np.float32)
    bk = np.asarray(bk, dtype=np.float32)
    wv = np.asarray(wv, dtype=np.float32)
    bv = np.asarray(bv, dtype=np.float32)
    wo = np.asarray(wo, dtype=np.float32)
    bo = np.asarray(bo, dtype=np.float32)

    scale = np.float32(1.0 / np.sqrt(D))
    wq_s = _round_f32r(wq * scale)
    bq_s = (bq * scale).astype(np.float32)

    wk_r = _round_f32r(wk)
    wv_r2 = _round_f32r(wv)
    wo_r = _round_f32r(wo)

    ones = np.zeros((SK, 2, 128), dtype=np.float32)
    ones[:, 0, 0:64] = 1.0
    ones[:, 1, 64:128] = 1.0
    ones = ones.reshape(SK, 2 * 128)
    z2 = np.zeros((SK, ET * 2 * 128), dtype=np.float32)

    nc = _get_program()
    in_maps = []
    for b in range(N_CORES):
        # [E, Sq] -> [chunk, partition, e-tile, col], contiguous per chunk.
        xT = _round_f32r(
            np.ascontiguousarray(
                x[b].T.reshape(ET, 128, NCHUNK, CHUNK).transpose(2, 1, 0, 3)
            )
        )
        yT = np.zeros((C, SKP), dtype=np.float32)
        yT[:, :SK] = y[b].T
        yT = _round_f32r(yT)
        in_maps.append(
            {
                "xT": xT,
                "yT": yT,
                "wq": wq_s,
                "bq": bq_s,
                "wk": wk_r,
                "bk": bk.astype(np.float32),
                "wv": wv_r2,
                "bv": bv.astype(np.float32),
                "wo": wo_r,
                "bo": bo,
                "ones": ones,
                "z2": z2,
            }
        )

    trace = bool(int(os.environ.get("KERNEL_TRACE", "0")))
    kwargs = {}
    if trace:
        tdir = os.environ.get("KERNEL_TRACE_DIR")
        if tdir:
            # Stale NTFFs from a previous run break neuron-profile's
            # one-json-per-model-index assumption.
            import glob as _glob
            import shutil as _shutil

            if os.path.isdir(tdir):
                _shutil.rmtree(tdir, ignore_errors=True)
            os.makedirs(tdir, exist_ok=True)
        kwargs = {"trace": True, "tmpdir": tdir}
    try:
        res = run_bass_kernel_spmd(nc, in_maps, list(range(N_CORES)), **kwargs)
    except Exception:
        # The axon-tunneled devices occasionally report a transient
        # NRT_EXEC_UNIT_UNRECOVERABLE; a retry on the same executable has
        # been observed to succeed.
        res = run_bass_kernel_spmd(nc, in_maps, list(range(N_CORES)), **kwargs)
    if trace:
        kernel.last_exec_time_ns = res.exec_time_ns
        kernel.last_results = res
    out = np.stack([res.results[b]["out"] for b in range(N_CORES)])
    return np.ascontiguousarray(out)


# revision 6
# speedup vs baseline: 1.1384x; 1.1384x over previous
"""Cross-attention kernel for Trainium2 (8 NeuronCores, data-parallel over batch).

Problem (hardcoded): B=8, Sq=4096, Sk=77, E=1024, C=768, H=16 heads, D=64.

    q = x @ wq + bq; k = y @ wk + bk; v = y @ wv + bv
    out = softmax(q k^T / sqrt(D)) v @ wo + bo

Sharding: batch element b -> core b. No collectives.

Per-core device pipeline (all matmuls contract over the SBUF partition dim):
  - Projection operands (x, y, wq, wk, wv, wo, oT) are bf16: same 1
    cycle/row PE rate as fp32r but half the HBM traffic, which is what
    gates the startup (the k/v-side weights + wq + the first x chunk must
    land before the first attention block). The attention core (qT, kT,
    exps, V-pairs) stays fp32r (11-bit mantissa) since it is on-chip.
  - Activations are kept feature-major: xT[E, Sq] is prepared host-side, so
    the wq-lhsT matmul chain produces qT[E, Sq] directly and per-head
    slices feed scores without any on-chip transpose.
  - kT[E, Sk] via lhsT=wk tiles; V[Sk, E] via lhsT=yT tiles, then re-packed
    on-chip into the zero-padded pair layout v2[Sk, et, {[v_even|0],
    [0|v_odd]}] (zeros DMA'd from a host constant, values written by
    strided DVE copies).
  - scores^T[Sk, q] = matmul(lhsT=kT head slice [64, 77], rhs=qT head
    slice); 1/sqrt(D) folded into wq/bq host-side. softmax without
    max-subtraction (scores are O(5), exp is safe in fp32): exp on ScalarE.
  - Attention output for a head PAIR accumulates into one [128, 512] PSUM
    tile (lhsT=[v_even|0] start, lhsT=[0|v_odd] stop): rows 0:64 even head,
    64:128 odd head - exactly the oT layout the final projection wants.
    Pair denominators accumulate the same way from ones-block lhsT into a
    second tile, arriving pre-broadcast per 64-partition half; DVE
    fast-approx reciprocal evicts them to SBUF, then ONE DVE multiply per
    pair evicts pav * recip -> oT (bf16), fusing normalization into the
    eviction.
  - out[q, E] row-major = matmul(lhsT=oT tiles [128, 128] bf16, rhs=wo
    tiles bf16), bias bo added during eviction.
"""

import os
from contextlib import ExitStack

import ml_dtypes
import numpy as np

import concourse.bass as bass
import concourse.tile as tile
from concourse import bacc, mybir
from concourse.bass_utils import run_bass_kernel_spmd

N_CORES = 8
SQ = 4096
SK = 77
SKP = 80  # SK padded: fp32r matmul dst free-size must be even
E = 1024
C = 768
H = 16
D = 64
CHUNK = 512
NCHUNK = SQ // CHUNK  # 8
ET = E // 128  # 8 e-tiles
CT = C // 128  # 6 c-tiles
F32 = mybir.dt.float32
F32R = mybir.dt.float32r
BF16 = mybir.dt.bfloat16

_PROGRAM = None


def _round_f32r(a: np.ndarray) -> np.ndarray:
    """Round fp32 to the fp32r format (11 mantissa bits, RNE)."""
    u = np.ascontiguousarray(a, dtype=np.float32).view(np.uint32).copy()
    u += np.uint32(0x7FF) + ((u >> np.uint32(12)) & np.uint32(1))
    u &= np.uint32(0xFFFFF000)
    return u.view(np.float32)


def _bf16(a: np.ndarray) -> np.ndarray:
    return np.ascontiguousarray(a, dtype=np.float32).astype(ml_dtypes.bfloat16)


def _build_program():
    nc = bacc.Bacc(
        "TRN2", target_bir_lowering=False, debug=False, num_devices=N_CORES
    )
    # xT pre-tiled host-side: [chunk, partition, e-tile, col] so each chunk's
    # SBUF tile is one contiguous 1MB DRAM read.
    xT_d = nc.dram_tensor(
        "xT", [NCHUNK, 128, ET, CHUNK], BF16, kind="ExternalInput"
    ).ap()
    yT_d = nc.dram_tensor("yT", [C, SKP], BF16, kind="ExternalInput").ap()
    wq_d = nc.dram_tensor("wq", [E, E], BF16, kind="ExternalInput").ap()
    bq_d = nc.dram_tensor("bq", [E], F32, kind="ExternalInput").ap()
    wk_d = nc.dram_tensor("wk", [C, E], BF16, kind="ExternalInput").ap()
    bk_d = nc.dram_tensor("bk", [E], F32, kind="ExternalInput").ap()
    wv_d = nc.dram_tensor("wv", [C, H * 64], BF16, kind="ExternalInput").ap()
    bv_d = nc.dram_tensor("bv", [H * 64], F32, kind="ExternalInput").ap()
    wo_d = nc.dram_tensor("wo", [E, E], BF16, kind="ExternalInput").ap()
    bo_d = nc.dram_tensor("bo", [E], F32, kind="ExternalInput").ap()
    # ones blocks for pair denominators: [:, 0, 0:64] = 1 and [:, 1, 64:128] = 1
    ones_d = nc.dram_tensor("ones", [SK, 2 * 128], F32R, kind="ExternalInput").ap()
    z2_d = nc.dram_tensor("z2", [SK, ET * 2 * 128], F32R, kind="ExternalInput").ap()
    out_d = nc.dram_tensor("out", [SQ, E], F32, kind="ExternalOutput").ap()

    with tile.TileContext(nc) as tc, ExitStack() as ctx:
        consts = ctx.enter_context(tc.tile_pool(name="consts", bufs=1))
        wq_sb = consts.tile([128, ET, E], BF16)
        wo_sb = consts.tile([128, ET, E], BF16)
        kT_sb = consts.tile([128, ET, SKP], F32R)
        v2_sb = consts.tile([SK, ET, 2, 128], F32R)
        ones_sb = consts.tile([SK, 2, 128], F32R)
        bq_sb = consts.tile([128, ET], F32)
        bk_sb = consts.tile([128, ET], F32)
        bv_sb = consts.tile([SK, H * 64], F32)
        bo_sb = consts.tile([128, E], F32)

        # k/v-side loads first: smallest data chain to the first PE op.
        nc.sync.dma_start(bk_sb[:], bk_d.rearrange("(t p) -> p t", p=128))
        nc.sync.dma_start(bv_sb[:], bv_d.partition_broadcast(SK))

        xT_pool = ctx.enter_context(tc.tile_pool(name="xT", bufs=2))
        qT_pool = ctx.enter_context(tc.tile_pool(name="qT", bufs=1))
        ps_q = ctx.enter_context(tc.tile_pool(name="ps_q", bufs=2, space="PSUM"))

        def load_xT(c):
            xT_sb = xT_pool.tile([128, ET, CHUNK], BF16, tag="xT")
            nc.sync.dma_start(xT_sb[:], xT_d[c])
            return xT_sb

        def emit_qt(xT_sb):
            qT_sb = qT_pool.tile([128, ET, CHUNK], F32R, tag="qT", name="qT_sb")
            for et in range(ET):
                ps = ps_q.tile([128, CHUNK], F32, tag="psq")
                for t in range(ET):
                    nc.tensor.matmul(
                        ps[:],
                        wq_sb[:, t, et * 128 : (et + 1) * 128],
                        xT_sb[:, t, :],
                        start=(t == 0),
                        stop=(t == ET - 1),
                    )
                nc.scalar.activation(
                    qT_sb[:, et, :],
                    ps[:],
                    mybir.ActivationFunctionType.Identity,
                    bias=bq_sb[:, et : et + 1],
                )
            return qT_sb

        # Phase 0: kT[E, Sk], V[Sk, E], and the padded pair layout v2.
        # V first (t-major, trailing the wv tile loads), then kT ets 0..3
        # t-major (4 PSUM slots), then ets 4..7 with wk fully resident.
        with tc.tile_pool(name="ph0", bufs=1) as ph0, tc.tile_pool(
            name="ph0ps", bufs=4, space="PSUM"
        ) as ph0ps:
            yT_sb = ph0.tile([128, CT, SKP], BF16)
            wk_sb = ph0.tile([128, CT, E], BF16)
            wv_sb = ph0.tile([128, CT, H * 64], BF16)
            v_sb = ph0.tile([SK, H * 64], F32R)
            yT_r = yT_d.rearrange("(t p) n -> p t n", p=128)
            wk_r = wk_d.rearrange("(t p) n -> p t n", p=128)
            wv_r = wv_d.rearrange("(t p) n -> p t n", p=128)
            nc.sync.dma_start(yT_sb[:], yT_r)
            for t in range(CT):
                nc.sync.dma_start(wv_sb[:, t, :], wv_r[:, t, :])
            for t in range(CT):
                nc.sync.dma_start(wk_sb[:, t, :], wk_r[:, t, :])
            # q-path loads queue behind the k/v side; QT(0) overlaps phase 0.
            wq_r = wq_d.rearrange("(t p) n -> p t n", p=128)
            for lo, hi in ((0, 4), (4, 8)):
                nc.sync.dma_start(wq_sb[:, lo:hi, :], wq_r[:, lo:hi, :])
            nc.sync.dma_start(bq_sb[:], bq_d.rearrange("(t p) -> p t", p=128))
            xT_cur = load_xT(0)
            nc.sync.dma_start(ones_sb[:], ones_d.rearrange("p (s n) -> p s n", s=2))
            nc.sync.dma_start(
                v2_sb[:], z2_d.rearrange("p (t s n) -> p t s n", t=ET, s=2)
            )
            nc.sync.dma_start(bo_sb[:], bo_d.partition_broadcast(128))

            psv = [
                ph0ps.tile([SK, CHUNK], F32, tag="ph0", name=f"psv{g}")
                for g in range(2)
            ]
            for t in range(CT):
                for g in range(2):
                    nc.tensor.matmul(
                        psv[g][:],
                        yT_sb[:, t, 0:SK],
                        wv_sb[:, t, g * CHUNK : (g + 1) * CHUNK],
                        start=(t == 0),
                        stop=(t == CT - 1),
                    )
            for g in range(2):
                nc.vector.tensor_tensor(
                    v_sb[:, g * CHUNK : (g + 1) * CHUNK],
                    psv[g][:],
                    bv_sb[:, g * CHUNK : (g + 1) * CHUNK],
                    mybir.AluOpType.add,
                )
                # Re-pack group g (heads 8g..8g+7, ets 4g..4g+3) into the
                # padded pair layout with two strided copies.
                e0 = 4 * g
                v_g = v_sb[:, g * CHUNK : (g + 1) * CHUNK].rearrange(
                    "p (e s d) -> p e s d", e=4, s=2
                )
                with nc.allow_low_precision(reason="fp32r pass-through copy"):
                    nc.vector.tensor_copy(
                        v2_sb[:, e0 : e0 + 4, 0, 0:64], v_g[:, :, 0, :]
                    )
                    nc.vector.tensor_copy(
                        v2_sb[:, e0 : e0 + 4, 1, 64:128], v_g[:, :, 1, :]
                    )
            for half in range(2):
                for et in range(4 * half, 4 * half + 4):
                    psk = ph0ps.tile([128, SKP], F32, tag="ph0", name="psk")
                    for t in range(CT):
                        nc.tensor.matmul(
                            psk[:],
                            wk_sb[:, t, et * 128 : (et + 1) * 128],
                            yT_sb[:, t, :],
                            start=(t == 0),
                            stop=(t == CT - 1),
                        )
                    nc.scalar.activation(
                        kT_sb[:, et, :],
                        psk[:],
                        mybir.ActivationFunctionType.Identity,
                        bias=bk_sb[:, et : et + 1],
                    )

            # QT(0) inside the ph0 scope so its matmuls fill the DMA tail.
            qT_sb0 = emit_qt(xT_cur)

        # Main loop pools.
        oT_pool = ctx.enter_context(tc.tile_pool(name="oT", bufs=2))
        rb_pool = ctx.enter_context(tc.tile_pool(name="rb", bufs=1))
        exps_pool = ctx.enter_context(tc.tile_pool(name="exps", bufs=4))
        outs_pool = ctx.enter_context(tc.tile_pool(name="outs", bufs=2))
        ps_s = ctx.enter_context(tc.tile_pool(name="ps_s", bufs=2, space="PSUM"))
        ps_av = ctx.enter_context(tc.tile_pool(name="ps_av", bufs=2, space="PSUM"))
        ps_den = ctx.enter_context(tc.tile_pool(name="ps_den", bufs=1, space="PSUM"))
        ps_f = ctx.enter_context(tc.tile_pool(name="ps_f", bufs=1, space="PSUM"))

        def emit_final_group(c, oT_sb, i):
            qt, n0 = i // 2, (i % 2) * CHUNK
            if i % 2 == 0:
                ps = ps_f.tile([128, CHUNK], F32, tag="psf")
            else:
                ps = ps_q.tile([128, CHUNK], F32, tag="psq", name="psfq")
            for t in range(ET):
                nc.tensor.matmul(
                    ps[:],
                    oT_sb[:, t, qt * 128 : (qt + 1) * 128],
                    wo_sb[:, t, n0 : n0 + CHUNK],
                    start=(t == 0),
                    stop=(t == ET - 1),
                )
            o_sb = outs_pool.tile([128, CHUNK], F32, tag="osb")
            nc.vector.tensor_tensor(
                o_sb[:], ps[:], bo_sb[:, n0 : n0 + CHUNK], mybir.AluOpType.add
            )
            r0 = c * CHUNK + qt * 128
            nc.sync.dma_start(out_d[r0 : r0 + 128, n0 : n0 + CHUNK], o_sb[:])

        # wo is first needed by final(0) during attention(1).
        wo_r = wo_d.rearrange("(t p) n -> p t n", p=128)
        for lo, hi in ((0, 4), (4, 8)):
            nc.sync.dma_start(wo_sb[:, lo:hi, :], wo_r[:, lo:hi, :])

        prev = None  # (c, oT_sb): chunk awaiting its final projection
        for c in range(NCHUNK):
            xT_sb = xT_cur
            if c + 1 < NCHUNK:
                xT_cur = load_xT(c + 1)
            qT_sb = qT_sb0 if c == 0 else emit_qt(xT_sb)

            # Attention for chunk c, interleaved with chunk c-1's output
            # projection. Per pair et: scores -> exp -> (finals cover the
            # exp latency) -> pair-packed attnV + pair-broadcast den ->
            # reciprocal to rb (DVE) + fused normalize-evict (DVE).
            oT_sb = oT_pool.tile([128, ET, CHUNK], BF16, tag="oT")
            rb_sb = rb_pool.tile([128, ET, CHUNK], F32, tag="rb")
            for et in range(ET):
                psa = ps_s.tile([SK, CHUNK], F32, tag="pss")
                psb = ps_s.tile([SK, CHUNK], F32, tag="pss")
                nc.tensor.matmul(
                    psa[:], kT_sb[0:64, et, 0:SK], qT_sb[0:64, et, :],
                    start=True, stop=True,
                )
                nc.tensor.matmul(
                    psb[:], kT_sb[64:128, et, 0:SK], qT_sb[64:128, et, :],
                    start=True, stop=True,
                )
                exa = exps_pool.tile([SK, CHUNK], F32R, tag="exps")
                exb = exps_pool.tile([SK, CHUNK], F32R, tag="exps")
                nc.scalar.activation(exa[:], psa[:], mybir.ActivationFunctionType.Exp)
                nc.scalar.activation(exb[:], psb[:], mybir.ActivationFunctionType.Exp)
                if prev is not None:
                    emit_final_group(prev[0], prev[1], et)
                pav = ps_av.tile([128, CHUNK], F32, tag="psav")
                den = ps_den.tile([128, CHUNK], F32, tag="psden")
                nc.tensor.matmul(
                    pav[:], v2_sb[:, et, 0, :], exa[:], start=True, stop=False
                )
                nc.tensor.matmul(
                    den[:], ones_sb[:, 0, :], exa[:], start=True, stop=False
                )
                nc.tensor.matmul(
                    pav[:], v2_sb[:, et, 1, :], exb[:], start=False, stop=True
                )
                nc.tensor.matmul(
                    den[:], ones_sb[:, 1, :], exb[:], start=False, stop=True
                )
                nc.vector.reciprocal_approx_fast(rb_sb[:, et, :], den[:])
                with nc.allow_low_precision(reason="bf16 oT feeds bf16 finals"):
                    nc.vector.tensor_tensor(
                        oT_sb[:, et, :],
                        pav[:],
                        rb_sb[:, et, :],
                        mybir.AluOpType.mult,
                    )
            prev = (c, oT_sb)
        # Tail: project the last chunk (already normalized at eviction).
        for i in range(8):
            emit_final_group(prev[0], prev[1], i)

    nc.compile()
    return nc


def _get_program():
    global _PROGRAM
    if _PROGRAM is None:
        _PROGRAM = _build_program()
    return _PROGRAM


def kernel(x, y, wq, bq, wk, bk, wv, bv, wo, bo):
    x = np.asarray(x, dtype=np.float32)
    y = np.asarray(y, dtype=np.float32)
    wq = np.asarray(wq, dtype=np.float32)
    bq = np.asarray(bq, dtype=np.float32)
    wk = np.asarray(wk, dtype=np.float32)
    bk = np.asarray(bk, dtype=np.float32)
    wv = np.asarray(wv, dtype=np.float32)
    bv = np.asarray(bv, dtype=np.float32)
    wo = np.asarray(wo, dtype=np.float32)
    bo = np.asarray(bo, dtype=np.float32)

    scale = np.float32(1.0 / np.sqrt(D))
    wq_s = _bf16(wq * scale)
    bq_s = (bq * scale).astype(np.float32)

    wk_b = _bf16(wk)
    wv_b = _bf16(wv)
    wo_b = _bf16(wo)

    ones = np.zeros((SK, 2, 128), dtype=np.float32)
    ones[:, 0, 0:64] = 1.0
    ones[:, 1, 64:128] = 1.0
    ones = ones.reshape(SK, 2 * 128)
    z2 = np.zeros((SK, ET * 2 * 128), dtype=np.float32)

    nc = _get_program()
    in_maps = []
    for b in range(N_CORES):
        # [E, Sq] -> [chunk, partition, e-tile, col], contiguous per chunk.
        xT = _bf16(
            np.ascontiguousarray(
                x[b].T.reshape(ET, 128, NCHUNK, CHUNK).transpose(2, 1, 0, 3)
            )
        )
        yT = np.zeros((C, SKP), dtype=np.float32)
        yT[:, :SK] = y[b].T
        yT = _bf16(yT)
        in_maps.append(
            {
                "xT": xT,
                "yT": yT,
                "wq": wq_s,
                "bq": bq_s,
                "wk": wk_b,
                "bk": bk.astype(np.float32),
                "wv": wv_b,
                "bv": bv.astype(np.float32),
                "wo": wo_b,
                "bo": bo,
                "ones": ones,
                "z2": z2,
            }
        )

    trace = bool(int(os.environ.get("KERNEL_TRACE", "0")))
    kwargs = {}
    if trace:
        tdir = os.environ.get("KERNEL_TRACE_DIR")
        if tdir:
            import shutil as _shutil

            if os.path.isdir(tdir):
                _shutil.rmtree(tdir, ignore_errors=True)
            os.makedirs(tdir, exist_ok=True)
        kwargs = {"trace": True, "tmpdir": tdir}
    try:
        res = run_bass_kernel_spmd(nc, in_maps, list(range(N_CORES)), **kwargs)
    except Exception:
        # The axon-tunneled devices occasionally report a transient
        # NRT_EXEC_UNIT_UNRECOVERABLE; a retry on the same executable has
        # been observed to succeed.
        res = run_bass_kernel_spmd(nc, in_maps, list(range(N_CORES)), **kwargs)
    if trace:
        kernel.last_exec_time_ns = res.exec_time_ns
        kernel.last_results = res
    out = np.stack([res.results[b]["out"] for b in range(N_CORES)])
    return np.ascontiguousarray(out)
